# revision 51
# baseline (speedup 1.0000x reference)
"""Block-causal GQA attention layer on 8 Trainium2 NeuronCores.

Sharding: 8 cores = batch(2) x head-group(4). Core c handles batch b=c//4 and
head group g=c%4 (q heads 4g..4g+3, kv head g). W_attn is column-sharded by
head group, W_proj row-sharded; each core computes a partial [T, C] output and
the host sums the 4 partials per batch element.

Per-core device pipeline (fp16 operands everywhere, f32 PSUM accumulation):
  B) x arrives pre-transposed from host (xt [C, T] fp16) so the QKV matmuls
     consume DMA'd xt tiles directly -- no PE transposes.  Software-pipelined
     over 16 t-chunks with a 1-chunk lag: stage A(i) = QKV matmuls + RMS
     stats (ACT square-accum, Sqrt, DVE reciprocal); stage B(i-1) =
     rs-premult (DVE/ACT), RoPE (Pool/DVE, negative-stride half-swap views,
     norm weights folded into host cos/sin tables) into a combined
     qkhat [t, 5*128] tile (4 q heads | k), then ONE XBAR DMA transpose per
     chunk lands qkT [d, c, t] (3D-out transpose: logical row = c*128+d).
     All tensors consumed by phase C are split by T-range (qkT/v per
     512-block, yT per (block, head)) so readers never pick up false deps
     on later writers.
  C+D) per (head, 512-wide T-block) group, software-pipelined one S-tile
     ahead so the PE never waits on ACT exp: scores sT = kT.T @ qT
     (block-causal lower tiles only), exp on ACT (scale=1/sqrt(d), bias=-1
     for fp16 headroom; softmax shift-invariant), staircase mask on diag
     tiles (DVE fp16 fast mode), PV matmul.  The softmax denominator is
     accumulated from exp tiles into pexA/pexB (Pool/DVE alternating) and
     reduced with ONE pair of ones-matmuls per group instead of a
     ones-matmul per tile (Ti=0 keeps per-tile den: cheaper).  Group
     finalization (den matmuls, reciprocal, yT normalize) is deferred one
     group so the PE never stalls on the pex chains.  Output-projection
     matmuls for T-block Ti-1 are woven one-per-iteration between
     score/PV pairs of Ti to keep the PE busy under ACT's exp stream;
     projection e-blocks close with a 2-deep stagger and ship one out-DMA
     per t-chunk.
"""

import numpy as np

import concourse.bacc as bacc
import concourse.bass as bass
import concourse.tile as tile
import concourse.mybir as mybir
from concourse.bass_utils import run_bass_kernel_spmd

P = 128
T = 2048
C = 2048
N_HEAD = 16
N_KV = 4
HD = 128          # head dim
HG = N_HEAD // N_KV  # heads per group = 4
BLOCK = 16
EPS = 1e-5
ROPE_BASE = 500000.0
QCOLS = HG * HD   # 512 q cols per core
JCOLS = QCOLS + 2 * HD  # 768 qkv cols per core
NT = T // P       # 16 t-chunks
NC16 = C // P     # 16 c-chunks
SCALE = 1.0 / float(np.sqrt(np.float32(HD)))

F32 = mybir.dt.float32
F16 = mybir.dt.float16
AF = mybir.ActivationFunctionType
ALU = mybir.AluOpType


def build_nc():
    nc = bacc.Bacc("TRN2", target_bir_lowering=False)

    xt = nc.dram_tensor("xt", [C, T], F16, kind="ExternalInput")
    wa = nc.dram_tensor("wa", [C, JCOLS], F16, kind="ExternalInput")
    wp = nc.dram_tensor("wp", [QCOLS, C], F16, kind="ExternalInput")
    csq = nc.dram_tensor("csq", [T, HD], F16, kind="ExternalInput")
    snq = nc.dram_tensor("snq", [T, HD], F16, kind="ExternalInput")
    csk = nc.dram_tensor("csk", [T, HD], F16, kind="ExternalInput")
    snk = nc.dram_tensor("snk", [T, HD], F16, kind="ExternalInput")
    dm1 = nc.dram_tensor("dm1", [P, P], F16, kind="ExternalInput")
    dm2 = nc.dram_tensor("dm2", [P, 256], F16, kind="ExternalInput")
    out = nc.dram_tensor("out", [T, C], F16, kind="ExternalOutput")

    half = HD // 2

    with tile.TileContext(nc) as tc:
        with tc.tile_pool(name="persist", bufs=1) as persist:
            ones16 = persist.tile([P, P], F16)
            nc.vector.memset(ones16, 1.0)
            dm1_sb = persist.tile([P, P], F16)
            dm2_sb = persist.tile([P, 256], F16)
            eps_sb = persist.tile([P, 1], F32)
            nc.vector.memset(eps_sb, EPS)
            negone = persist.tile([P, 1], F32)
            nc.vector.memset(negone, -1.0)

            # split by T-range so readers don't pick up false deps on
            # later writers (tile dep tracking is per-tensor).
            # qkTs[i][:, c, t]: c = q heads 0..3, c=4 is k.
            qkTs = [
                persist.tile([P, HG + 1, 512], F16, name=f"qkT{i}")
                for i in range(4)
            ]
            vs = [
                persist.tile([P, 4, HD], F16, name=f"v{i}") for i in range(4)
            ]
            yTs = [
                [
                    persist.tile([P, 512], F16, name=f"yT{i}h{hh}")
                    for hh in range(HG)
                ]
                for i in range(4)
            ]
            wp_sb = persist.tile([P, HG, C], F16)

            # ---------------- Phase B (pipelined, lag 1) -----------------
            # bwork is opened OUTSIDE the B scope: its tiles (rope chain,
            # last-chunk PSUM copies) are still being read by DMA
            # transposes when phase C starts; an open pool doesn't fence,
            # so C's first instructions no longer stall on the B tail.
            bwork_cm = tc.tile_pool(name="bwork", bufs=4)
            bwork = bwork_cm.__enter__()
            bper_cm = tc.tile_pool(name="bper", bufs=1)
            bper = bper_cm.__enter__()
            with (
                tc.tile_pool(name="wts", bufs=1) as wts,
                tc.tile_pool(name="xstream", bufs=4) as xstream,
                tc.tile_pool(name="psB_qa", bufs=2, space="PSUM") as psB_qa,
                tc.tile_pool(name="psB_qb", bufs=2, space="PSUM") as psB_qb,
            ):
                # xt streamed in 8 pieces of 2 t-chunks (512 B rows)
                xt_r = xt[:].rearrange("(ci p) t -> p ci t", p=P)
                x_pieces = {}

                def dma_xt(pc, quarters=2):
                    # ci-sliced DMAs per piece: shorter exclusive holds
                    # on the DMA engines so XBAR transposes interleave
                    xb = xstream.tile([P, NC16, 256], F16, tag="xt")
                    q = NC16 // quarters
                    for hf in range(quarters):
                        nc.sync.dma_start(
                            xb[:, hf * q:(hf + 1) * q, :],
                            xt_r[:, hf * q:(hf + 1) * q,
                                 pc * 256:(pc + 1) * 256],
                        )
                    x_pieces[pc] = xb

                wa_tiles = []

                def dma_wa(ci):
                    wa_ci = wts.tile([P, JCOLS], F16, name=f"wa{ci}")
                    nc.sync.dma_start(wa_ci, wa[ci * P:(ci + 1) * P, :])
                    wa_tiles.append(wa_ci)

                dma_xt(0)
                for ci in range(NC16):
                    dma_wa(ci)
                dma_xt(1)

                # rope tables (needed only from B1(0), ~chunk 2's time)
                tabs = {}
                for nm, dr in (("csq", csq), ("snq", snq),
                               ("csk", csk), ("snk", snk)):
                    tt = bper.tile([P, NT, HD], F16, name=f"tab_{nm}")
                    nc.sync.dma_start(tt, dr[:].rearrange("(c p) d -> p c d", p=P))
                    tabs[nm] = tt
                dma_xt(2)
                dma_xt(3)
                nc.sync.dma_start(dm1_sb, dm1[:])
                nc.sync.dma_start(dm2_sb, dm2[:])

                st = {}  # chunk -> state

                def stageB1(j):
                    """premult by rs, v copy, rope for chunk j (ACT/DVE/Pool).

                    qkhat holds rotated q (4 heads) then k, h-major; ONE
                    XBAR transpose per chunk lands [d, c, t] directly
                    (3D-out transpose: logical row = c*128 + d)."""
                    s = st[j]
                    qa_ps, qb_ps, rs = s["qa"], s["qb"], s["rs"]
                    qrs = bwork.tile([P, QCOLS + HD], F16, tag="qrs")
                    for hh in range(HG):
                        nc.vector.tensor_scalar_mul(
                            qrs[:, hh * HD:(hh + 1) * HD],
                            qa_ps[:, hh * HD:(hh + 1) * HD],
                            rs[:, hh:hh + 1],
                        )
                    nc.scalar.mul(
                        qrs[:, QCOLS:QCOLS + HD], qb_ps[:, 0:HD], rs[:, HG:HG + 1]
                    )
                    nc.scalar.copy(vs[j // 4][:, j % 4, :], qb_ps[:, HD:2 * HD])

                    qswp = bass.AP(
                        tensor=qrs.tensor,
                        offset=qrs.offset + half,
                        ap=[qrs.ap[0], [HD, HG], [-half, 2], [1, half]],
                    )
                    csq_c = tabs["csq"][:, j, :]
                    csq_b = bass.AP(
                        tensor=csq_c.tensor,
                        offset=csq_c.offset,
                        ap=[csq_c.ap[0], [0, HG], [1, HD]],
                    )
                    snq_c = tabs["snq"][:, j, :]
                    snq_b = bass.AP(
                        tensor=snq_c.tensor,
                        offset=snq_c.offset,
                        ap=[snq_c.ap[0], [0, HG], [half, 2], [1, half]],
                    )
                    t1q = bwork.tile([P, QCOLS], F16, tag="t1q")
                    nc.gpsimd.tensor_tensor(
                        t1q.rearrange("p (h e) -> p h e", h=HG),
                        qrs[:, 0:QCOLS].rearrange("p (h e) -> p h e", h=HG),
                        csq_b,
                        ALU.mult,
                    )
                    t2q = bwork.tile([P, QCOLS], F16, tag="t2q")
                    nc.vector.tensor_tensor(
                        t2q.rearrange("p (h s e) -> p h s e", h=HG, s=2),
                        qswp,
                        snq_b,
                        ALU.mult,
                    )
                    qkhat = bwork.tile([P, (HG + 1) * HD], F16, tag="qkhat")
                    nc.vector.tensor_tensor(
                        qkhat[:, 0:QCOLS], t1q, t2q, ALU.add
                    )

                    kswp = bass.AP(
                        tensor=qrs.tensor,
                        offset=qrs.offset + QCOLS + half,
                        ap=[qrs.ap[0], [-half, 2], [1, half]],
                    )
                    t1k = bwork.tile([P, HD], F16, tag="t1k")
                    nc.gpsimd.tensor_tensor(
                        t1k, qrs[:, QCOLS:QCOLS + HD], tabs["csk"][:, j, :], ALU.mult
                    )
                    t2k = bwork.tile([P, HD], F16, tag="t2k")
                    nc.vector.tensor_tensor(
                        t2k.rearrange("p (s e) -> p s e", s=2),
                        kswp,
                        tabs["snk"][:, j, :].rearrange("p (s e) -> p s e", s=2),
                        ALU.mult,
                    )
                    nc.vector.tensor_tensor(
                        qkhat[:, QCOLS:QCOLS + HD], t1k, t2k, ALU.add
                    )
                    s["qkhat"] = qkhat

                def stageB2(j):
                    """One XBAR DMA transpose qkhat -> qkT chunk."""
                    s = st.pop(j)
                    tq0 = (j % 4) * P
                    nc.sync.dma_start(
                        qkTs[j // 4][:, :, tq0:tq0 + P],
                        s["qkhat"],
                        transpose=True,
                    )

                for i in range(NT):
                    if True:
                        pc, tc2 = divmod(i, 2)
                        if i % 2 == 1 and pc + 4 <= 7:
                            dma_xt(pc + 4)
                        if 8 <= i < 12:
                            for e in (2 * (i - 8), 2 * (i - 8) + 1):
                                nc.sync.dma_start(
                                    wp_sb[:, :, e * 256:(e + 1) * 256],
                                    wp[:, e * 256:(e + 1) * 256].rearrange(
                                        "(h d) e -> d h e", d=P
                                    ),
                                )
                        s = {}
                        st[i] = s
                        xb = x_pieces[pc]
                        qa_ps = psB_qa.tile([P, QCOLS], F32, tag="qa")
                        # padded to a full PSUM bank so the next chunk's
                        # accumulation doesn't share a bank with this one
                        qb_full = psB_qb.tile([P, 512], F32, tag="qb")
                        qb_ps = qb_full[:, 0:2 * HD]
                        for ci in range(NC16):
                            lhs = xb[:, ci, tc2 * P:(tc2 + 1) * P]
                            nc.tensor.matmul(
                                qa_ps, lhs, wa_tiles[ci][:, 0:QCOLS],
                                start=(ci == 0), stop=(ci == NC16 - 1),
                            )
                            nc.tensor.matmul(
                                qb_ps, lhs, wa_tiles[ci][:, QCOLS:JCOLS],
                                start=(ci == 0), stop=(ci == NC16 - 1),
                            )
                        s["qa"], s["qb"] = qa_ps, qb_ps

                        if i >= 1 and i - 1 <= NT - 3:
                            stageB1(i - 1)

                        # RMS stats for chunk i
                        ss = bwork.tile([P, HG + 1], F32, tag="ss")
                        for hh in range(HG + 1):
                            src = (
                                s["qa"][:, hh * HD:(hh + 1) * HD]
                                if hh < HG
                                else s["qb"][:, 0:HD]
                            )
                            sq = bwork.tile([P, HD], F16, tag="sq")
                            nc.scalar.activation(
                                sq, src, AF.Square, accum_out=ss[:, hh:hh + 1]
                            )
                        rt = bwork.tile([P, HG + 1], F32, tag="rt")
                        nc.scalar.activation(
                            rt, ss, AF.Sqrt, bias=eps_sb, scale=1.0 / HD
                        )
                        rs = bwork.tile([P, HG + 1], F32, tag="rs")
                        nc.vector.reciprocal(rs, rt)
                        s["rs"] = rs

                        if i >= 1 and i - 1 <= NT - 3:
                            stageB2(i - 1)
                for j in (NT - 2, NT - 1):
                    stageB1(j)
                    stageB2(j)

            # ---------------- Phase C+D interleaved ----------------------
            OFFS = [0, 128, 256, 256]
            with (
                tc.tile_pool(name="cwork", bufs=4) as cwork,
                tc.tile_pool(name="cwork2", bufs=2) as cwork2,
                tc.tile_pool(name="dout", bufs=3) as dout,
                tc.tile_pool(name="ps_sc", bufs=2, space="PSUM") as ps_sc,
                tc.tile_pool(name="ps_o", bufs=2, space="PSUM") as ps_o,
                tc.tile_pool(name="ps_yt", bufs=2, space="PSUM") as ps_yt,
                tc.tile_pool(name="ps_den", bufs=2, space="PSUM") as ps_den,
            ):
                def proj_gen(Tb, fine_dma=False):
                    """Yield after each PE matmul of T-block Tb's projection.

                    e-blocks pipelined depth-2 so the accumulation-closing
                    (h=3) matmul of one e-block is deferred until after the
                    h=0..2 matmuls of the next (hides yT-normalize latency
                    at the final flush)."""
                    yTb = yTs[Tb]  # list of per-head tensors
                    pend = []  # [(o_ps, part, e)], closed 2 e-blocks later
                    osb = {}

                    def close(po, ppart, pe):
                        nc.tensor.matmul(
                            po,
                            yTb[HG - 1][:, ppart * P:(ppart + 1) * P],
                            wp_sb[:, HG - 1, pe * 512:(pe + 1) * 512],
                            start=False, stop=True,
                        )

                    def drain(po, ppart, pe):
                        nc.vector.tensor_copy(
                            osb[ppart][:, pe * 512:(pe + 1) * 512], po
                        )
                        pt0 = (4 * Tb + ppart) * P
                        if fine_dma:
                            nc.sync.dma_start(
                                out[pt0:pt0 + P, pe * 512:(pe + 1) * 512],
                                osb[ppart][:, pe * 512:(pe + 1) * 512],
                            )
                        elif pe == 3:
                            nc.sync.dma_start(out[pt0:pt0 + P, :], osb[ppart])

                    for part in range(4):
                        t0 = part * P
                        osb[part] = dout.tile([P, C], F16, tag="osb", name="osb_t")
                        for e in range(4):
                            o_ps = ps_o.tile([P, 512], F32, tag="o")
                            for h in range(HG - 1):
                                nc.tensor.matmul(
                                    o_ps,
                                    yTb[h][:, t0:t0 + P],
                                    wp_sb[:, h, e * 512:(e + 1) * 512],
                                    start=(h == 0), stop=False,
                                )
                                yield
                            if len(pend) == 2:
                                close(*pend[0])
                                yield
                                drain(*pend.pop(0))
                            pend.append((o_ps, part, e))
                    for k in range(2):
                        close(*pend[k])
                        yield
                        drain(*pend[k])

                def finalize(g):
                    """Deferred group end: den matmul(s), reciprocal, yT."""
                    if g is None:
                        return
                    if g["pex"] is not None:
                        pexA, pexB = g["pex"]
                        nc.tensor.matmul(
                            g["den"], ones16, pexA, start=True, stop=False
                        )
                        nc.tensor.matmul(
                            g["den"], ones16, pexB, start=False, stop=True
                        )
                    denr = cwork2.tile([P, 512], F32, tag="denr")
                    scr = cwork2.tile([P, 512], F32, tag="scr")
                    nc.vector.reciprocal_approx_accurate(denr, g["den"], scr)
                    nc.vector.tensor_tensor(
                        yTs[g["ti"]][g["h"]], g["yt"], denr, ALU.mult,
                    )

                prev_group = None
                filler = None
                fill_left = 0

                for Ti in range(4):
                    tt0 = Ti * 512
                    nS = 4 * Ti + 4
                    if Ti >= 1:
                        filler = proj_gen(Ti - 1, fine_dma=True)
                        fill_left = 64
                    n_iters = HG * nS

                    for h in range(HG):
                        yt_ps = ps_yt.tile([P, 512], F32, tag="yt")
                        den_ps = ps_den.tile([P, 512], F32, tag="den")
                        use_pex = Ti >= 1
                        if use_pex:
                            pexA = cwork2.tile([P, 512], F16, tag="pexA")
                            pexB = cwork2.tile([P, 512], F16, tag="pexB")
                        ex_tiles = {}

                        def emit_sc_exp(S):
                            r = S - 4 * Ti
                            off = OFFS[r] if r >= 0 else 0
                            sc_ps = ps_sc.tile([P, 512], F32, tag="sc")
                            nc.tensor.matmul(
                                sc_ps[:, off:512],
                                qkTs[S // 4][:, HG, (S % 4) * P:(S % 4 + 1) * P],
                                qkTs[Ti][:, h, off:512],
                                start=True, stop=True,
                            )
                            ex = cwork.tile([P, 512], F16, tag="ex")
                            nc.scalar.activation(
                                ex[:, off:512], sc_ps[:, off:512], AF.Exp,
                                bias=negone, scale=SCALE,
                            )
                            if r >= 0:
                                if r < 3:
                                    nc.vector.tensor_tensor(
                                        ex[:, r * P:(r + 1) * P],
                                        ex[:, r * P:(r + 1) * P],
                                        dm1_sb, ALU.mult,
                                    )
                                else:
                                    nc.vector.tensor_tensor(
                                        ex[:, 256:512], ex[:, 256:512],
                                        dm2_sb, ALU.mult,
                                    )
                            ex_tiles[S] = (ex, off)

                        emit_sc_exp(0)
                        for S in range(nS):
                            if S + 1 < nS:
                                emit_sc_exp(S + 1)
                            if S == 1 and prev_group is not None:
                                finalize(prev_group)
                                prev_group = None
                            ex, off = ex_tiles.pop(S)
                            nc.tensor.matmul(
                                yt_ps[:, off:512],
                                vs[S // 4][:, S % 4, :],
                                ex[:, off:512],
                                start=(S == 0), stop=(S == nS - 1),
                            )
                            if not use_pex:
                                nc.tensor.matmul(
                                    den_ps[:, off:512], ones16, ex[:, off:512],
                                    start=(S == 0), stop=(S == nS - 1),
                                )
                            else:
                                # halves -> two accumulators; engines
                                # alternate per op to split chain load
                                early = S < nS // 2
                                eng = nc.gpsimd if S % 2 == 0 else nc.vector
                                dst = pexA if early else pexB
                                if S == 0 or S == nS // 2:
                                    eng.tensor_copy(dst[:, off:512], ex[:, off:512])
                                else:
                                    eng.tensor_tensor(
                                        dst[:, off:512], dst[:, off:512],
                                        ex[:, off:512], ALU.add,
                                    )
                            # weave projection matmuls of Ti-1
                            if filler is not None and fill_left > 0:
                                it_done = h * nS + S
                                it_left = n_iters - it_done
                                want = -(-fill_left // max(it_left, 1))
                                if h == 0 and S < 2:
                                    want = 0
                                for _ in range(min(want, fill_left)):
                                    try:
                                        next(filler)
                                        fill_left -= 1
                                    except StopIteration:
                                        fill_left = 0
                                        break
                        prev_group = {
                            "yt": yt_ps, "den": den_ps, "h": h, "ti": Ti,
                            "pex": (pexA, pexB) if use_pex else None,
                        }
                    # drain any leftover filler at Ti end
                    if filler is not None:
                        while fill_left > 0:
                            try:
                                next(filler)
                                fill_left -= 1
                            except StopIteration:
                                fill_left = 0
                        for _ in filler:
                            pass
                        filler = None

                finalize(prev_group)
                prev_group = None
                # final projection for Ti=3
                for _ in proj_gen(3, fine_dma=True):
                    pass
            bper_cm.__exit__(None, None, None)
            bwork_cm.__exit__(None, None, None)

    nc.finalize()
    return nc


def _host_tables(q_norm_w, k_norm_w):
    """RoPE cos/sin tables in [t, d] layout with norm weights folded in."""
    half = HD // 2
    inv_freq = (
        1.0 / (ROPE_BASE ** (np.arange(0, half, dtype=np.float32) / half))
    ).astype(np.float32)
    ang = np.arange(T, dtype=np.float32)[:, None] * inv_freq[None, :]  # [T, half]
    cos = np.cos(ang).astype(np.float32)
    sin = np.sin(ang).astype(np.float32)
    cos2 = np.concatenate([cos, cos], axis=1)           # [T, 128]
    sin2 = np.concatenate([-sin, sin], axis=1)          # [T, 128]
    csq = np.ascontiguousarray(cos2 * q_norm_w[None, :], dtype=np.float32)
    snq = np.ascontiguousarray(sin2 * q_norm_w[None, :], dtype=np.float32)
    csk = np.ascontiguousarray(cos2 * k_norm_w[None, :], dtype=np.float32)
    snk = np.ascontiguousarray(sin2 * k_norm_w[None, :], dtype=np.float32)
    return csq, snq, csk, snk


def _host_masks():
    idx = np.arange(P)
    stair = (idx[None, :] // BLOCK >= idx[:, None] // BLOCK).astype(np.float16)
    dm1 = stair
    dm2 = np.concatenate([np.zeros((P, P), np.float16), stair], axis=1)
    return np.ascontiguousarray(dm1), np.ascontiguousarray(dm2)


def _make_in_maps(x, W_attn, W_proj, q_norm_w, k_norm_w):
    csq, snq, csk, snk = (
        a.astype(np.float16) for a in _host_tables(q_norm_w, k_norm_w)
    )
    dm1, dm2 = _host_masks()
    xts = [np.ascontiguousarray(x[b].T).astype(np.float16) for b in range(2)]
    was, wps = [], []
    for g in range(4):
        wa_core = np.concatenate(
            [
                W_attn[:, g * QCOLS:(g + 1) * QCOLS],
                W_attn[:, C + g * HD:C + (g + 1) * HD],
                W_attn[:, C + N_KV * HD + g * HD:C + N_KV * HD + (g + 1) * HD],
            ],
            axis=1,
        ).astype(np.float16)
        was.append(np.ascontiguousarray(wa_core))
        wps.append(
            np.ascontiguousarray(
                W_proj[g * QCOLS:(g + 1) * QCOLS, :].astype(np.float16)
            )
        )
    in_maps = []
    for core in range(8):
        b, g = divmod(core, 4)
        in_maps.append(
            {
                "xt": xts[b], "wa": was[g], "wp": wps[g],
                "csq": csq, "snq": snq, "csk": csk, "snk": snk,
                "dm1": dm1, "dm2": dm2,
            }
        )
    return in_maps


_nc_cache = None


def kernel(x, W_attn, W_proj, q_norm_w, k_norm_w):
    global _nc_cache
    x = np.asarray(x, dtype=np.float32)
    W_attn = np.asarray(W_attn, dtype=np.float32)
    W_proj = np.asarray(W_proj, dtype=np.float32)
    q_norm_w = np.asarray(q_norm_w, dtype=np.float32)
    k_norm_w = np.asarray(k_norm_w, dtype=np.float32)
    B = x.shape[0]

    in_maps = _make_in_maps(x, W_attn, W_proj, q_norm_w, k_norm_w)

    if _nc_cache is None:
        _nc_cache = build_nc()
    res = run_bass_kernel_spmd(_nc_cache, in_maps, core_ids=list(range(8)))

    out = np.zeros((B, T, C), dtype=np.float32)
    for core in range(8):
        b = core // 4
        out[b] += res.results[core]["out"].astype(np.float32)
    return out
